# revision 1
# baseline (speedup 1.0000x reference)
"""DistillLoss CQ ColBERT (MaxSim + KLDiv) Trainium2 Bass kernel.

Full inputs in, scalar loss out. Shards the batch dim B=128 across 8
NeuronCores (16 b's per core); each core computes its local MaxSim for
both d_cq (student) and d_orig (teacher), the per-b KL terms, and the
host sums the per-core partials (the "all-reduce") and divides by B.

Dataflow per (b, t[cq|orig]): d[*,b] is cast-DMA'd to bf16 in natural
layout [128(k%128), 8n, 2(k//128), 128d]; per-k sumsq via DVE
scalar_tensor_tensor accum; w = mask/max(||d||,eps); scale d*w (bf16);
PE-transpose (bf16) to put D on partitions; PSUM->SBUF copy; bf16
matmuls q_hi/q_lo (split fp32 q) x dT col-packed 4 units per PSUM tile
with a rank-4 matmul adding the -9999 mask offsets; one batched
reduce_max over all 128 partitions; tiny on-device KL at the end.

Hardcoded problem shape:
  q_reps [128, 32, 128] f32, d_cq/d_orig [8, 128, 256, 128] f32,
  d_mask [8, 128, 256] i32, labels [128, 8] f32 (unused by reference).
"""

import numpy as np
import ml_dtypes

import concourse.bass as bass
import concourse.bacc as bacc_mod
import concourse.mybir as mybir
import concourse.tile as tile
from concourse.bass_utils import run_bass_kernel_spmd

B, N, Lq, Ld, D = 128, 8, 32, 256, 128
NCORES = 8
BL = B // NCORES  # 16 b's per core
NEG = -9999.0
F32 = mybir.dt.float32
BF16 = mybir.dt.bfloat16

# tuning flags
GP_SCALE_MOD = 1      # n % GP_SCALE_MOD != 0 -> scale on gpsimd (else DVE)
NEWTON_SQRT = False    # refine ACT sqrt (65536-ULP budget) w/ one Newton step
Q_SPLIT = False       # q_hi/q_lo bf16 split (2 matmuls, fp32-exact q side)


def _build_program():
    nc = bacc_mod.Bacc("TRN2", target_bir_lowering=False, debug=False)

    q_in = nc.declare_dram_parameter("q", [BL, Lq, D], F32, isOutput=False)
    dcq_in = nc.declare_dram_parameter("dcq", [N, BL, Ld, D], F32, isOutput=False)
    dor_in = nc.declare_dram_parameter("dorig", [N, BL, Ld, D], F32, isOutput=False)
    maskf_in = nc.declare_dram_parameter("maskf", [N * BL, Ld], F32, isOutput=False)
    # offs[u, b, 0:256]=offs[n=u, b]; offs[u, b, 256:512]=offs[n=u+4, b]
    offs_in = nc.declare_dram_parameter("offs", [4, BL, 2 * Ld], BF16, isOutput=False)
    ident_in = nc.declare_dram_parameter("ident", [128, 128], BF16, isOutput=False)
    e4t_in = nc.declare_dram_parameter("e4t", [128, 4], F32, isOutput=False)
    e4_in = nc.declare_dram_parameter("e4", [4, 128], BF16, isOutput=False)
    klb_out = nc.declare_dram_parameter("klb", [BL, 1], F32, isOutput=True)

    AF = mybir.ActivationFunctionType
    ALU = mybir.AluOpType

    with tile.TileContext(nc) as tc:
        with (
            tc.tile_pool(name="const", bufs=1) as const,
            tc.tile_pool(name="dtiles", bufs=4) as dtiles,
            tc.tile_pool(name="scratch", bufs=6) as scratch,
            tc.tile_pool(name="wpool", bufs=6) as wpool,
            tc.tile_pool(name="dscp", bufs=6) as dscp,
            tc.tile_pool(name="dtsb", bufs=8) as dtsb,
            tc.tile_pool(name="klp", bufs=1) as klp,
            tc.tile_pool(name="ps_tr", bufs=5, space="PSUM") as ps_tr,
            tc.tile_pool(name="ps_sc", bufs=2, space="PSUM") as ps_sc,
            tc.tile_pool(name="ps_sm", bufs=1, space="PSUM") as ps_sm,
            tc.tile_pool(name="dram", bufs=1, space="DRAM") as dram,
        ):
            # ---------- constants ----------
            ident = const.tile([128, 128], BF16)
            nc.sync.dma_start(out=ident, in_=ident_in[:])
            e4t = const.tile([128, 4], F32)
            nc.sync.dma_start(out=e4t, in_=e4t_in[:])
            e4 = const.tile([4, 128], BF16)
            nc.sync.dma_start(out=e4, in_=e4_in[:])
            offs_sb = const.tile([4, BL, 2 * Ld], BF16)
            nc.sync.dma_start(out=offs_sb, in_=offs_in[:])
            mask_nat = const.tile([128, 256], BF16)
            nc.gpsimd.dma_start(out=mask_nat, in_=maskf_in[:])  # casts to bf16

            # ---------- maskT: [128(p=k%128), 2(c=k//128), 128(nb)] ----------
            mT_ps = ps_tr.tile([128, 2, 128], BF16, tag="tp")
            mview = mask_nat.rearrange("q (p c) -> q p c", c=2)
            for c in range(2):
                nc.tensor.transpose(mT_ps[:, c, :], mview[:, :, c], ident)
            maskT = const.tile([128, 2, 128], F32)
            nc.vector.tensor_copy(maskT, mT_ps)

            # ---------- q-hat T -> bf16 hi/lo: [128(dd), BL*Lq] ----------
            qhi = const.tile([128, BL * Lq], BF16)
            qlo = const.tile([128, BL * Lq], BF16)
            for i in range(4):  # 4 b's per tile -> [128(bq), 128(dd)]
                qn = scratch.tile([128, 128], F32, tag="qnat")
                nc.sync.dma_start(
                    out=qn,
                    in_=q_in[4 * i:4 * i + 4].rearrange("b q d -> (b q) d"),
                )
                qss = wpool.tile([128, 1], F32, tag="qss")
                sq = scratch.tile([128, 128], F32, tag="qsq")
                nc.vector.scalar_tensor_tensor(
                    out=sq, in0=qn, scalar=1.0, in1=qn,
                    op0=ALU.mult, op1=ALU.mult, accum_out=qss,
                )
                nrm = wpool.tile([128, 1], F32, tag="qnrm")
                nc.scalar.activation(out=nrm, in_=qss, func=AF.Sqrt)
                if NEWTON_SQRT:
                    r = wpool.tile([128, 1], F32, tag="qr")
                    nc.vector.reciprocal(out=r, in_=nrm)
                    p1 = wpool.tile([128, 1], F32, tag="qp1")
                    nc.vector.tensor_mul(out=p1, in0=qss, in1=r)
                    nc.vector.tensor_add(out=p1, in0=p1, in1=nrm)
                    nc.vector.tensor_scalar_mul(out=nrm, in0=p1, scalar1=0.5)
                nc.vector.tensor_scalar_max(out=nrm, in0=nrm, scalar1=1e-12)
                rinv = wpool.tile([128, 1], F32, tag="qrinv")
                nc.vector.reciprocal(out=rinv, in_=nrm)
                qsc = scratch.tile([128, 128], F32, tag="qsc")
                nc.vector.tensor_scalar_mul(out=qsc, in0=qn, scalar1=rinv)
                qhn = scratch.tile([128, 128], BF16, tag="qhn")
                nc.vector.tensor_copy(qhn, qsc)
                qln = scratch.tile([128, 128], BF16, tag="qln")
                nc.vector.tensor_sub(out=qln, in0=qsc, in1=qhn)
                for src, dst in ((qhn, qhi), (qln, qlo)):
                    qt_ps = ps_tr.tile([128, 128], BF16, tag="tp")
                    nc.tensor.transpose(qt_ps, src, ident)
                    nc.vector.tensor_copy(dst[:, 128 * i:128 * (i + 1)], qt_ps)

            # rm_all[p=(u,q), b, t, h] row maxes; n = 4h + u
            rm_all = const.tile([128, BL, 2, 2], F32)

            # ---------- main loop ----------
            for b in range(BL):
                for t in range(2):
                    d_in = dcq_in if t == 0 else dor_in
                    d_nat = dtiles.tile([128, N, 2, 128], BF16)
                    # p = k//2, c = k%2: per-partition source runs are two
                    # whole 512B rows (1KB contiguous) -> one 3D-balanceable
                    # SWDGE DMA per (b,t) that also casts f32 -> bf16
                    nc.gpsimd.dma_start(
                        out=d_nat.rearrange("p n c d -> p n (c d)"),
                        in_=d_in[:, b].rearrange("n (p c) d -> p n (c d)", c=2))
                    # norms: ss[p, c, n] = sum_d d^2
                    ss = wpool.tile([128, 2, N], F32, tag="ss")
                    # wait-absorber: a cheap DVE op that reads the DMA'd tile
                    # (advancing DVE's view of the DMA sem) and writes into a
                    # DVE-only-history scratch corner; the narrow-wait-budget
                    # STT ops below then carry only their same-engine wait
                    sq0 = scratch.tile([128, 128], F32, tag="dsq")
                    nc.vector.tensor_copy(sq0[:, 0:2], d_nat[:, 0, 0, 0:2])
                    for n in range(N):
                        for c in range(2):
                            sq = sq0 if (n == 0 and c == 0) else                                 scratch.tile([128, 128], F32, tag="dsq")
                            nc.vector.scalar_tensor_tensor(
                                out=sq, in0=d_nat[:, n, c, :], scalar=1.0,
                                in1=d_nat[:, n, c, :],
                                op0=ALU.mult, op1=ALU.mult,
                                accum_out=ss[:, c, n:n + 1],
                            )
                    # w = maskT / max(sqrt(ss), eps)
                    w = wpool.tile([128, 2, N], F32, tag="w")
                    nc.scalar.activation(out=w, in_=ss, func=AF.Sqrt)
                    if NEWTON_SQRT:
                        r = wpool.tile([128, 2, N], F32, tag="wr")
                        nc.vector.reciprocal(out=r, in_=w)
                        nc.vector.tensor_mul(out=r, in0=ss, in1=r)
                        nc.vector.tensor_add(out=w, in0=w, in1=r)
                        nc.vector.tensor_scalar_mul(out=w, in0=w, scalar1=0.5)
                    nc.vector.tensor_scalar_max(out=w, in0=w, scalar1=1e-12)
                    nc.vector.reciprocal(out=w, in_=w)
                    mslice = maskT.rearrange("p c (n b) -> p c n b", b=BL)[:, :, :, b]
                    nc.vector.tensor_mul(out=w, in0=w, in1=mslice)

                    # scale + transpose + copy, paired (u, u+4) -> [128, 512]
                    dT_pairs = []
                    for u in range(4):
                        dT_ps = ps_tr.tile([128, 512], BF16, tag="tp")
                        for h in range(2):
                            n = 4 * h + u
                            dsc = dscp.tile([128, 2, 128], BF16, tag="dsc")
                            eng = nc.vector if (n % GP_SCALE_MOD == 0) else nc.gpsimd
                            for c in range(2):
                                eng.tensor_scalar_mul(
                                    out=dsc[:, c, :], in0=d_nat[:, n, c, :],
                                    scalar1=w[:, c, n:n + 1],
                                )
                            for c in range(2):
                                nc.tensor.transpose(
                                    dT_ps[:, 256 * h + 128 * c:
                                          256 * h + 128 * (c + 1)],
                                    dsc[:, c, :], ident,
                                )
                        dT = dtsb.tile([128, 512], BF16, tag="dt")
                        nc.scalar.copy(out=dT, in_=dT_ps)
                        dT_pairs.append(dT)

                    # scores: offs rank-4 first (fewest fresh deps), then
                    # 4 col-packed unit matmuls accumulate on top
                    sc_ps = ps_sc.tile([128, 512], F32, tag="scps")
                    nc.tensor.matmul(
                        sc_ps[:, :], e4, offs_sb[:, b, :],
                        start=True, stop=False,
                        skip_group_check=True,
                    )
                    for u in range(4):
                        nc.tensor.matmul(
                            sc_ps[32 * u:32 * (u + 1), :],
                            qhi[:, 32 * b:32 * (b + 1)],
                            dT_pairs[u],
                            start=False, stop=False,
                            tile_position=(0, 32 * u),
                            skip_group_check=True,
                        )
                        if Q_SPLIT:
                            nc.tensor.matmul(
                                sc_ps[32 * u:32 * (u + 1), :],
                                qlo[:, 32 * b:32 * (b + 1)],
                                dT_pairs[u],
                                start=False, stop=(u == 3),
                                tile_position=(0, 32 * u),
                                skip_group_check=True,
                            )
                    nc.vector.reduce_max(
                        out=rm_all[:, b, t, :],
                        in_=sc_ps.rearrange("p (h k) -> p h k", h=2),
                        axis=mybir.AxisListType.X,
                    )

            # ---------- sum over q (partition blocks) ----------
            sc_sm = ps_sm.tile([4, BL * 2 * 2], F32)
            nc.tensor.matmul(
                sc_sm, e4t, rm_all.rearrange("p b t h -> p (b t h)"),
                start=True, stop=True,
            )
            sc_sb = klp.tile([4, BL * 2 * 2], F32)
            nc.scalar.copy(out=sc_sb, in_=sc_sm)
            # repartition [4(u), b t h] -> [16(b), t h u] via DRAM bounce
            dbounce = dram.tile([4, BL, 2, 2], F32)
            nc.sync.dma_start(out=dbounce, in_=sc_sb.rearrange(
                "u (b t h) -> u b t h", b=BL, t=2))
            klin = klp.tile([BL, 2, 2, 4], F32)
            nc.sync.dma_start(
                out=klin, in_=dbounce.rearrange("u b t h -> b t h u"))

            # ---------- KL ----------
            ls = []
            exs = []
            zs = []
            for t in range(2):
                st = klin[:, t]  # [16, 2, 4]; n = 4h + u
                mxn = klp.tile([BL, 1], F32, tag=f"mx{t}")
                nc.vector.tensor_reduce(
                    out=mxn, in_=st, axis=mybir.AxisListType.XY,
                    op=ALU.max, negate=True,
                )
                ex = klp.tile([BL, 8], F32, tag=f"ex{t}")
                nc.scalar.activation(
                    out=ex, in_=st.rearrange("b h u -> b (h u)"),
                    func=AF.Exp, bias=mxn, scale=1.0,
                )
                z = klp.tile([BL, 1], F32, tag=f"z{t}")
                nc.vector.tensor_reduce(
                    out=z, in_=ex, axis=mybir.AxisListType.X, op=ALU.add)
                lz = klp.tile([BL, 1], F32, tag=f"lz{t}")
                nc.scalar.activation(out=lz, in_=z, func=AF.Ln)
                lsm = klp.tile([BL, 8], F32, tag=f"lsm{t}")
                nc.vector.tensor_scalar(
                    out=lsm, in0=st.rearrange("b h u -> b (h u)"),
                    scalar1=mxn, scalar2=lz,
                    op0=ALU.add, op1=ALU.subtract,
                )
                ls.append(lsm)
                exs.append(ex)
                zs.append(z)
            rz = klp.tile([BL, 1], F32)
            nc.vector.reciprocal(out=rz, in_=zs[1])
            diff = klp.tile([BL, 8], F32)
            nc.vector.tensor_tensor(
                out=diff, in0=ls[1], in1=ls[0], op=ALU.subtract)
            terms = klp.tile([BL, 8], F32)
            nc.vector.scalar_tensor_tensor(
                out=terms, in0=exs[1], scalar=rz, in1=diff,
                op0=ALU.mult, op1=ALU.mult,
            )
            klb = klp.tile([BL, 1], F32)
            nc.vector.tensor_reduce(
                out=klb, in_=terms, axis=mybir.AxisListType.X, op=ALU.add)
            nc.sync.dma_start(out=klb_out[:], in_=klb)

    nc.compile()
    return nc


_PROG = None


def _get_program():
    global _PROG
    if _PROG is None:
        _PROG = _build_program()
    return _PROG


def _host_consts():
    ident = np.eye(128, dtype=np.float32).astype(ml_dtypes.bfloat16)
    e4t = np.zeros((128, 4), dtype=np.float32)
    for j in range(4):
        e4t[32 * j:32 * (j + 1), j] = 1.0
    e4 = e4t.T.astype(ml_dtypes.bfloat16)
    return ident, e4t, e4


def make_in_maps(q_reps, d_cq, d_orig, d_mask):
    ident, e4t, e4 = _host_consts()
    in_maps = []
    for c in range(NCORES):
        sl = slice(c * BL, (c + 1) * BL)
        maskf = d_mask[:, sl].reshape(N * BL, Ld).astype(np.float32)
        # offs[n, b, k] = (mask - 1) * 9999; paired [u, b, (h k)] n = 4h+u
        # dT k-order within a pair tile: pos = 256h + 128c + p, k = 2p + c
        offs_nbk = ((maskf - 1.0) * (-NEG)).reshape(N, BL, 128, 2)
        offs = np.empty((4, BL, 2 * Ld), dtype=np.float32)
        for u in range(4):
            for h in range(2):
                for c in range(2):
                    offs[u, :, 256 * h + 128 * c:256 * h + 128 * (c + 1)] =                         offs_nbk[4 * h + u, :, :, c]
        in_maps.append({
            "q": np.ascontiguousarray(q_reps[sl]),
            "dcq": np.ascontiguousarray(d_cq[:, sl]),
            "dorig": np.ascontiguousarray(d_orig[:, sl]),
            "maskf": maskf,
            "offs": offs.astype(ml_dtypes.bfloat16),
            "ident": ident,
            "e4t": e4t,
            "e4": e4,
        })
    return in_maps


def kernel(q_reps, d_cq, d_orig, d_mask, labels):
    nc = _get_program()
    in_maps = make_in_maps(q_reps, d_cq, d_orig, d_mask)
    res = run_bass_kernel_spmd(nc, in_maps, list(range(NCORES)))
    total = 0.0
    for c in range(NCORES):
        total += float(np.asarray(res.results[c]["klb"], dtype=np.float64).sum())
    return np.float32(total / B)

